# revision 25
# baseline (speedup 1.0000x reference)
"""Trainium2 Bass kernel for nn_CategoryMultiplier.

out[b, s, :] = inputs[b, s, :] * (emb_table[categories[b, s]] if
               categories[b, s] != 0 else 1.0)

Sharding: pure data parallel over batch. 8 cores x 16 batches each.

Precision: the grading gate is rel_err < 2e-2; fp16 end-to-end keeps the
max relative error at ~7e-4 while halving every HBM stream. Host converts
f32 -> fp16 in and back out.

Category-sorted pairing (the big byte saver): the host sorts each core's
8192 positions by category, so equal-category runs (~8 long for 1000
vocab) become contiguous slots, and pads odd runs so every within-
partition PAIR of slots shares one category. The kernel then gathers ONE
table row per pair -- 4.7MB instead of 9.4MB of gather traffic -- and the
DVE multiply broadcasts each row over its pair with a stride-0 AP dim.
Worst case padding is bounded (<=1000 odd categories), so the padded
slot count is fixed at N_S = 9216 (72 per partition). Dummy slots carry
x = 0 and are dropped on the host-side unpermute. Total DMA traffic per
core: x 9.4 + y 9.4 + rows 4.7 = 23.6MB, vs 25.3MB unsorted (the DMA
engines are the roofline at ~22.3GB/s x 16).

Gather desc-gen parallelism: the dma_gather ucode dispatches on
`cpu_id / 2 == queue_num`, i.e. each SWDGE queue is served by a distinct
Q7 core pair and the pairs race ahead across instructions. Chunks rotate
across queue_num 0..3. The idx stream is wrapped in 16 partitions and
replicated across the 8 groups so every queue's pair sees it.

Device layout: slots are partition-major (partition p holds slots
p*72 .. p*72+71). dma_gather's fixed dst layout dst[i%128, i//128] is
reconciled by permuting the pair-index array on the host (pure layout
prep). Deep io prefetch covers the ~20us gpsimd library-load window.

Padding rows (category 0 -> multiplier 1.0): baked into the host fp16
table copy (row 0 = ones); index 0 is semantically dead.
"""

import numpy as np

import concourse.bass as bass
import concourse.bacc as bacc
import concourse.mybir as mybir
import concourse.tile as tile
from concourse.bass_utils import run_bass_kernel_spmd

# Problem shape (hardcoded per harness contract).
B, S, D = 128, 512, 512
VOCAB = 1000
N_CORES = 8
B_LOC = B // N_CORES            # 16 batches per core
N = B_LOC * S                   # 8192 positions per core
P = 128                         # SBUF partitions

N_S = 9216                      # padded slots per core (worst case 9192)
C_S = N_S // P                  # 72 slots per partition
NPAIR = C_S // 2                # 36 pairs per partition
# 8 chunks: exactly 8 SWDGE gathers (matches the 8-sem SWDGE pool, so the
# sem-rotation machinery never kicks in) and 17 HWDGE DMAs (minimal
# rotation pressure on the global 8-sem HWDGE pool). Head taper: small
# first chunks finish desc-gen fast after the library load, starting the
# mul -> store chain ~8us earlier.
PAIR_CHUNKS = [2, 4, 5, 5, 5, 5, 5, 5]
assert sum(PAIR_CHUNKS) == NPAIR
PAIR_MAX = max(PAIR_CHUNKS)
N_Q = 4                         # SWDGE queues / Q7 pairs used for gathers

F16 = mybir.dt.float16
I16 = mybir.dt.int16


def _build_nc():
    nc = bacc.Bacc("TRN2", target_bir_lowering=False, debug=False,
                   num_swdge_queues=N_Q)

    x = nc.dram_tensor("x", [N_S, D], F16, kind="ExternalInput")
    catsp = nc.dram_tensor("catsp", [P, NPAIR * 8], I16, kind="ExternalInput")
    table = nc.dram_tensor("table", [VOCAB, D], F16, kind="ExternalInput")
    y = nc.dram_tensor("y", [N_S, D], F16, kind="ExternalOutput")

    xr = x[:].rearrange("(p c) d -> p (c d)", p=P)     # [128, C_S*D]
    yr = y[:].rearrange("(p c) d -> p (c d)", p=P)

    # Issue the GPSIMD ucode library load BEFORE the TileContext so the
    # IRAM load overlaps Tile's own prologue barrier.
    from concourse.library_config import mlp
    nc.gpsimd.load_library(mlp)

    with tile.TileContext(nc) as tc:
        with (
            tc.tile_pool(name="const", bufs=1) as const_pool,
            tc.tile_pool(name="io", bufs=8) as io_pool,
            tc.tile_pool(name="gat", bufs=8) as gat_pool,
        ):
            cats_t = const_pool.tile([P, NPAIR * 8], I16)
            nc.scalar.dma_start(out=cats_t[:], in_=catsp[:])

            ppos = 0
            for ci, pch in enumerate(PAIR_CHUNKS):
                # one gathered row per pair
                n_idx = pch * P
                g_t = gat_pool.tile([P, PAIR_MAX * D], F16, tag="g")
                nc.gpsimd.dma_gather(
                    out_ap=g_t[:, :pch * D].rearrange("p (t d) -> p t d",
                                                      t=pch),
                    in_ap=table[:],
                    idxs_ap=cats_t[:, ppos * 8:(ppos + pch) * 8],
                    num_idxs=n_idx,
                    num_idxs_reg=n_idx,
                    elem_size=D,
                    queue_num=ci % N_Q,
                )

                lo, hi = ppos * 2 * D, (ppos + pch) * 2 * D
                x_t = io_pool.tile([P, 2 * PAIR_MAX * D], F16, tag="x")
                nc.sync.dma_start(out=x_t[:, :pch * 2 * D], in_=xr[:, lo:hi])

                # x[p, pair, k, :] *= row[p, pair, :] broadcast over k (step 0)
                xa = x_t[:]
                ga = g_t[:]
                x4 = bass.AP(xa.tensor, xa.offset,
                             [xa.ap[0], (2 * D, pch), (D, 2), (1, D)])
                g4 = bass.AP(ga.tensor, ga.offset,
                             [ga.ap[0], (D, pch), (0, 2), (1, D)])
                nc.vector.tensor_mul(out=x4, in0=x4, in1=g4)
                # All y-stores on scalar: mixing them into sync's in-order
                # queue head-of-line-blocks the x prefetch stream behind the
                # first store's mul dependency.
                nc.scalar.dma_start(out=yr[:, lo:hi], in_=x_t[:, :pch * 2 * D])
                ppos += pch

    nc.compile()
    return nc


_NC = None


def _get_nc():
    global _NC
    if _NC is None:
        _NC = _build_nc()
    return _NC


def _sort_pad(c):
    """Sort positions by category and pad so every within-partition pair of
    slots shares one category.

    Returns (slot_pos[N_S] int64 with -1 for dummy slots,
             pair_cats[P, NPAIR] int16)."""
    order = np.argsort(c, kind="stable")
    counts = np.bincount(c, minlength=VOCAB)
    padded = counts + (counts & 1)
    pstart = np.zeros(VOCAB + 1, dtype=np.int64)
    np.cumsum(padded, out=pstart[1:])
    bstart = np.zeros(VOCAB + 1, dtype=np.int64)
    np.cumsum(counts, out=bstart[1:])
    # slot index for each sorted element
    within = np.arange(N, dtype=np.int64) - np.repeat(bstart[:-1], counts)
    slots = np.repeat(pstart[:-1], counts) + within
    slot_pos = np.full(N_S, -1, dtype=np.int64)
    slot_pos[slots] = order
    slot_cat = np.zeros(N_S, dtype=np.int16)
    slot_cat[:pstart[-1]] = np.repeat(
        np.arange(VOCAB, dtype=np.int16), padded)
    pair_cats = slot_cat[0::2].reshape(P, NPAIR)
    return slot_pos, pair_cats


def _permute_pair_cats(pair_cats):
    """dma_gather idx stream: stream index s = pair_col*128 + p holds
    pair_cats[p, pair_col]; wrap (s at [s%16, s//16]) and replicate."""
    npairs = P * NPAIR
    a = np.ascontiguousarray(pair_cats.T).reshape(npairs)
    return np.ascontiguousarray(np.tile(a.reshape(npairs // 16, 16).T, (8, 1)))


def _shard_inputs(inputs, categories, emb_table):
    tab = np.array(emb_table, dtype=np.float16)
    tab[0, :] = np.float16(1.0)            # padding row -> multiplier 1.0
    in_maps = []
    shard_meta = []
    for i in range(N_CORES):
        xs = np.asarray(
            inputs[i * B_LOC:(i + 1) * B_LOC], dtype=np.float16
        ).reshape(N, D)
        c = categories[i * B_LOC:(i + 1) * B_LOC].reshape(N).astype(np.int64)
        slot_pos, pair_cats = _sort_pad(c)
        xdev = np.zeros((N_S, D), dtype=np.float16)
        valid = slot_pos >= 0
        xdev[valid] = xs[slot_pos[valid]]
        in_maps.append({"x": xdev, "catsp": _permute_pair_cats(pair_cats),
                        "table": tab})
        shard_meta.append((slot_pos, valid))
    return in_maps, shard_meta


def kernel(inputs, categories, mask_positions=None, emb_table=None, **_):
    """Full (unsharded) inputs in, full output out. mask_positions unused."""
    nc = _get_nc()
    in_maps, shard_meta = _shard_inputs(inputs, categories, emb_table)
    res = run_bass_kernel_spmd(nc, in_maps, list(range(N_CORES)))
    out = np.empty((B, S, D), dtype=np.float32)
    for i in range(N_CORES):
        slot_pos, valid = shard_meta[i]
        ydev = res.results[i]["y"].reshape(N_S, D)
        yfull = np.empty((N, D), dtype=np.float32)
        yfull[slot_pos[valid]] = ydev[valid].astype(np.float32)
        out[i * B_LOC:(i + 1) * B_LOC] = yfull.reshape(B_LOC, S, D)
    return out


# revision 28
# speedup vs baseline: 1.0595x; 1.0595x over previous
"""Trainium2 Bass kernel for nn_CategoryMultiplier.

out[b, s, :] = inputs[b, s, :] * (emb_table[categories[b, s]] if
               categories[b, s] != 0 else 1.0)

Sharding: data parallel, 8192 positions per core. Cores start from
contiguous 16-batch blocks; a few positions are then exchanged between
cores to balance padding (see below) -- the host fully controls the
position->slot mapping, so this is pure layout prep.

Precision: the grading gate is rel_err < 2e-2; fp16 end-to-end keeps the
max relative error at ~7e-4 while halving every HBM stream. Host converts
f32 -> fp16 in and back out.

Category-sorted pairing (the big byte saver): the host sorts each core's
positions by category, so equal-category runs become contiguous slots,
and pads odd runs so every within-partition PAIR of slots shares one
category. The kernel gathers ONE table row per pair and the DVE multiply
broadcasts each row over its pair with a stride-0 AP dim. Dummy slots
carry x = 0 and are dropped on the host-side unpermute.

Adaptive slot count: the padded slot count N_S depends on the input (one
pad slot per odd-count category, plus partition alignment). The kernel is
compiled lazily for the exact N_S the inputs need (cached per size)
instead of the 9216 worst case. Cross-core balancing first moves single
positions of categories that are odd in both the source and destination
core: such a move cuts the source's slot need by 2 and leaves the
destination's unchanged, pulling outlier cores under the next 256-slot
alignment boundary. Typical result: N_S = 8704 (68 slots/partition),
i.e. 23.1MB of DMA per core -- the 16 DMA engines at ~22.3GB/s each are
the roofline.

Chunking: at most 8 SWDGE gathers (matches Tile's global 8-sem SWDGE pool
so sem rotation never kicks in) and 17 HWDGE DMAs. Head taper ([2,4,5...]
pairs) starts the mul->store chain right after the ~19us GPSIMD library
load. Gathers rotate queue_num 0..3: each SWDGE queue is served by its
own Q7 core pair (ucode dispatches on cpu_id/2 == queue_num), so four
descriptor generators run concurrently. x-loads stay alone on sync and
y-stores alone on scalar: a store (which waits on compute) placed in the
prefetch engine's in-order queue would head-of-line-block the x stream.

Padding rows (category 0 -> multiplier 1.0): baked into the host fp16
table copy (row 0 = ones); index 0 is semantically dead.
"""

import numpy as np

import concourse.bass as bass
import concourse.bacc as bacc
import concourse.mybir as mybir
import concourse.tile as tile
from concourse.bass_utils import run_bass_kernel_spmd

# Problem shape (hardcoded per harness contract).
B, S, D = 128, 512, 512
VOCAB = 1000
N_CORES = 8
B_LOC = B // N_CORES            # 16 batches per core
N = B_LOC * S                   # 8192 positions per core
P = 128                         # SBUF partitions
N_S_MAX = 9216                  # worst case: 8192 + 1000 odd cats, aligned
N_Q = 4                         # SWDGE queues / Q7 pairs used for gathers

F16 = mybir.dt.float16
I16 = mybir.dt.int16


def _pair_chunks(npair):
    """Head-tapered chunk list: [2, 4, then <=5s], at most 8 chunks."""
    rest = npair - 6
    assert 0 < rest <= 30, npair
    n_tail = -(-rest // 5)
    chunks = [2, 4]
    for i in range(n_tail):
        chunks.append(rest // n_tail + (1 if i < rest % n_tail else 0))
    assert sum(chunks) == npair and len(chunks) <= 8 and max(chunks) <= 5
    return chunks


def _build_nc(n_s):
    c_s = n_s // P                 # slots per partition (even)
    npair = c_s // 2
    pair_chunks = _pair_chunks(npair)
    pair_max = max(pair_chunks)

    nc = bacc.Bacc("TRN2", target_bir_lowering=False, debug=False,
                   num_swdge_queues=N_Q)

    x = nc.dram_tensor("x", [n_s, D], F16, kind="ExternalInput")
    catsp = nc.dram_tensor("catsp", [P, npair * 8], I16, kind="ExternalInput")
    table = nc.dram_tensor("table", [VOCAB, D], F16, kind="ExternalInput")
    y = nc.dram_tensor("y", [n_s, D], F16, kind="ExternalOutput")

    xr = x[:].rearrange("(p c) d -> p (c d)", p=P)     # [128, c_s*D]
    yr = y[:].rearrange("(p c) d -> p (c d)", p=P)

    # Issue the GPSIMD ucode library load BEFORE the TileContext so the
    # IRAM load overlaps Tile's own prologue barrier.
    from concourse.library_config import mlp
    nc.gpsimd.load_library(mlp)

    with tile.TileContext(nc) as tc:
        with (
            tc.tile_pool(name="const", bufs=1) as const_pool,
            tc.tile_pool(name="io", bufs=8) as io_pool,
            tc.tile_pool(name="gat", bufs=8) as gat_pool,
        ):
            cats_t = const_pool.tile([P, npair * 8], I16)
            nc.scalar.dma_start(out=cats_t[:], in_=catsp[:])

            ppos = 0
            for ci, pch in enumerate(pair_chunks):
                # one gathered row per pair
                n_idx = pch * P
                g_t = gat_pool.tile([P, pair_max * D], F16, tag="g")
                nc.gpsimd.dma_gather(
                    out_ap=g_t[:, :pch * D].rearrange("p (t d) -> p t d",
                                                      t=pch),
                    in_ap=table[:],
                    idxs_ap=cats_t[:, ppos * 8:(ppos + pch) * 8],
                    num_idxs=n_idx,
                    num_idxs_reg=n_idx,
                    elem_size=D,
                    queue_num=ci % N_Q,
                )

                lo, hi = ppos * 2 * D, (ppos + pch) * 2 * D
                x_t = io_pool.tile([P, 2 * pair_max * D], F16, tag="x")
                nc.sync.dma_start(out=x_t[:, :pch * 2 * D], in_=xr[:, lo:hi])

                # x[p, pair, k, :] *= row[p, pair, :] broadcast over k (step 0)
                xa = x_t[:]
                ga = g_t[:]
                x4 = bass.AP(xa.tensor, xa.offset,
                             [xa.ap[0], (2 * D, pch), (D, 2), (1, D)])
                g4 = bass.AP(ga.tensor, ga.offset,
                             [ga.ap[0], (D, pch), (0, 2), (1, D)])
                nc.vector.tensor_mul(out=x4, in0=x4, in1=g4)
                nc.scalar.dma_start(out=yr[:, lo:hi],
                                    in_=x_t[:, :pch * 2 * D])
                ppos += pch

    nc.compile()
    return nc


_NC_CACHE = {}
_LAST_NS = N_S_MAX


def _get_nc(n_s=None):
    if n_s is None:
        n_s = _LAST_NS
    if n_s not in _NC_CACHE:
        _NC_CACHE[n_s] = _build_nc(n_s)
    return _NC_CACHE[n_s]


def _core_need(counts, npos):
    """Slots needed: positions + one pad per odd-count category."""
    return npos + int((counts & 1).sum())


def _balance_positions(cats):
    """Per-core global position lists, starting from contiguous blocks.

    Phase 1 -- parity pairing: for each category, cores where its count is
    odd are paired up and one position moves between each pair, making both
    counts even. Each such move removes two pad slots globally (the mover's
    core loses a position AND a pad; the receiver gains a position and
    loses a pad). Only categories with an odd GLOBAL count keep one odd
    core (~vocab/2 pads total instead of ~vocab/2 per core).

    Phase 2 -- need balancing: moving a position of a category that is odd
    in the source flips it even there (-2 slots) and odd at the destination
    (+2 slots), a pure transfer of slot need; use it to level the cores.
    """
    counts = np.stack([np.bincount(cats[i * N:(i + 1) * N],
                                   minlength=VOCAB) for i in range(N_CORES)])
    # per-core per-category global-position stacks (O(1) moves)
    by_cat = []
    for i in range(N_CORES):
        c = cats[i * N:(i + 1) * N]
        order = np.argsort(c, kind="stable") + i * N
        bnd = np.zeros(VOCAB + 1, dtype=np.int64)
        np.cumsum(counts[i], out=bnd[1:])
        by_cat.append([list(order[bnd[v]:bnd[v + 1]]) for v in range(VOCAB)])
    sizes = [N] * N_CORES

    def move(v, src, dst):
        by_cat[dst][v].append(by_cat[src][v].pop())
        counts[src][v] -= 1
        counts[dst][v] += 1
        sizes[src] -= 1
        sizes[dst] += 1

    for v in range(VOCAB):
        odd = [i for i in range(N_CORES) if counts[i][v] & 1]
        for k in range(0, len(odd) - 1, 2):
            a, b = odd[k], odd[k + 1]
            src, dst = (a, b) if sizes[a] >= sizes[b] else (b, a)
            move(v, src, dst)

    needs = [_core_need(counts[i], sizes[i]) for i in range(N_CORES)]
    for _ in range(2000):
        src = int(np.argmax(needs))
        dst = int(np.argmin(needs))
        if needs[src] - needs[dst] <= 2:
            break
        odd_src = np.flatnonzero(counts[src] & 1)
        if len(odd_src) == 0:
            break
        move(int(odd_src[0]), src, dst)
        needs[src] = _core_need(counts[src], sizes[src])
        needs[dst] = _core_need(counts[dst], sizes[dst])

    def ns_for(need):
        sp = -(-need // P)
        return (sp + (sp & 1)) * P

    pos = [np.array([p for v in range(VOCAB) for p in by_cat[i][v]],
                    dtype=np.int64) for i in range(N_CORES)]
    n_s = min(max(ns_for(n) for n in needs), N_S_MAX)
    return pos, n_s


def _sort_pad(c_local, n_s):
    """Sort a core's positions by category and pad so every within-partition
    pair of slots shares one category.

    Returns (slot_idx[n_s] int64 local indices with -1 for dummy slots,
             pair_cats[P, n_s//256] int16)."""
    npos = len(c_local)
    order = np.argsort(c_local, kind="stable")
    counts = np.bincount(c_local, minlength=VOCAB)
    padded = counts + (counts & 1)
    pstart = np.zeros(VOCAB + 1, dtype=np.int64)
    np.cumsum(padded, out=pstart[1:])
    bstart = np.zeros(VOCAB + 1, dtype=np.int64)
    np.cumsum(counts, out=bstart[1:])
    within = np.arange(npos, dtype=np.int64) - np.repeat(bstart[:-1], counts)
    slots = np.repeat(pstart[:-1], counts) + within
    slot_idx = np.full(n_s, -1, dtype=np.int64)
    slot_idx[slots] = order
    slot_cat = np.zeros(n_s, dtype=np.int16)
    slot_cat[:pstart[-1]] = np.repeat(
        np.arange(VOCAB, dtype=np.int16), padded)
    pair_cats = slot_cat[0::2].reshape(P, n_s // 256)
    return slot_idx, pair_cats


def _permute_pair_cats(pair_cats):
    """dma_gather idx stream: stream index s = pair_col*128 + p holds
    pair_cats[p, pair_col]; wrap (s at [s%16, s//16]) and replicate."""
    npairs = pair_cats.size
    a = np.ascontiguousarray(pair_cats.T).reshape(npairs)
    return np.ascontiguousarray(np.tile(a.reshape(npairs // 16, 16).T, (8, 1)))


def _shard_inputs(inputs, categories, emb_table):
    global _LAST_NS
    tab = np.array(emb_table, dtype=np.float16)
    tab[0, :] = np.float16(1.0)            # padding row -> multiplier 1.0
    cats = np.asarray(categories).reshape(B * S).astype(np.int64)
    xall = np.asarray(inputs, dtype=np.float16).reshape(B * S, D)
    pos, n_s = _balance_positions(cats)
    _LAST_NS = n_s
    in_maps = []
    shard_meta = []
    for i in range(N_CORES):
        slot_idx, pair_cats = _sort_pad(cats[pos[i]], n_s)
        valid = slot_idx >= 0
        slot_pos = np.full(n_s, -1, dtype=np.int64)
        slot_pos[valid] = pos[i][slot_idx[valid]]   # global position ids
        xdev = np.zeros((n_s, D), dtype=np.float16)
        xdev[valid] = xall[slot_pos[valid]]
        in_maps.append({"x": xdev, "catsp": _permute_pair_cats(pair_cats),
                        "table": tab})
        shard_meta.append((slot_pos, valid))
    return in_maps, shard_meta


def kernel(inputs, categories, mask_positions=None, emb_table=None, **_):
    """Full (unsharded) inputs in, full output out. mask_positions unused."""
    in_maps, shard_meta = _shard_inputs(inputs, categories, emb_table)
    nc = _get_nc(_LAST_NS)
    res = run_bass_kernel_spmd(nc, in_maps, list(range(N_CORES)))
    out = np.empty((B * S, D), dtype=np.float32)
    for i in range(N_CORES):
        slot_pos, valid = shard_meta[i]
        ydev = res.results[i]["y"].reshape(_LAST_NS, D)
        out[slot_pos[valid]] = ydev[valid].astype(np.float32)
    return out.reshape(B, S, D)


# revision 29
# speedup vs baseline: 1.0625x; 1.0028x over previous
"""Trainium2 Bass kernel for nn_CategoryMultiplier.

out[b, s, :] = inputs[b, s, :] * (emb_table[categories[b, s]] if
               categories[b, s] != 0 else 1.0)

Sharding: data parallel, 8192 positions per core. Cores start from
contiguous 16-batch blocks; a few positions are then exchanged between
cores to balance padding (see below) -- the host fully controls the
position->slot mapping, so this is pure layout prep.

Precision: the grading gate is rel_err < 2e-2; fp16 end-to-end keeps the
max relative error at ~7e-4 while halving every HBM stream. Host converts
f32 -> fp16 in and back out.

Category-sorted pairing (the big byte saver): the host sorts each core's
positions by category, so equal-category runs become contiguous slots,
and pads odd runs so every within-partition PAIR of slots shares one
category. The kernel gathers ONE table row per pair and the DVE multiply
broadcasts each row over its pair with a stride-0 AP dim. Dummy slots
carry x = 0 and are dropped on the host-side unpermute.

Adaptive slot count: the padded slot count N_S depends on the input (one
pad slot per odd-count category, plus partition alignment). The kernel is
compiled lazily for the exact N_S the inputs need (cached per size)
instead of the 9216 worst case. Cross-core balancing first moves single
positions of categories that are odd in both the source and destination
core: such a move cuts the source's slot need by 2 and leaves the
destination's unchanged, pulling outlier cores under the next 256-slot
alignment boundary. Typical result: N_S = 8704 (68 slots/partition),
i.e. 23.1MB of DMA per core -- the 16 DMA engines at ~22.3GB/s each are
the roofline.

Chunking: at most 8 SWDGE gathers (matches Tile's global 8-sem SWDGE pool
so sem rotation never kicks in) and 17 HWDGE DMAs. Head taper ([2,4,5...]
pairs) starts the mul->store chain right after the ~19us GPSIMD library
load. Gathers rotate queue_num 0..3: each SWDGE queue is served by its
own Q7 core pair (ucode dispatches on cpu_id/2 == queue_num), so four
descriptor generators run concurrently. x-loads stay alone on sync and
y-stores alone on scalar: a store (which waits on compute) placed in the
prefetch engine's in-order queue would head-of-line-block the x stream.

Padding rows (category 0 -> multiplier 1.0): baked into the host fp16
table copy (row 0 = ones); index 0 is semantically dead.
"""

import numpy as np

import concourse.bass as bass
import concourse.bacc as bacc
import concourse.mybir as mybir
import concourse.tile as tile
from concourse.bass_utils import run_bass_kernel_spmd

# Problem shape (hardcoded per harness contract).
B, S, D = 128, 512, 512
VOCAB = 1000
N_CORES = 8
B_LOC = B // N_CORES            # 16 batches per core
N = B_LOC * S                   # 8192 positions per core
P = 128                         # SBUF partitions
N_S_MAX = 9216                  # worst case: 8192 + 1000 odd cats, aligned
N_Q = 4                         # SWDGE queues / Q7 pairs used for gathers

F16 = mybir.dt.float16
I16 = mybir.dt.int16


def _pair_chunks(npair):
    """Head-tapered chunk list: [2, 4, then <=5s], at most 8 chunks."""
    rest = npair - 6
    assert 0 < rest <= 30, npair
    n_tail = -(-rest // 5)
    chunks = [2, 4]
    for i in range(n_tail):
        chunks.append(rest // n_tail + (1 if i < rest % n_tail else 0))
    assert sum(chunks) == npair and len(chunks) <= 8 and max(chunks) <= 5
    return chunks


def _build_nc(n_s):
    c_s = n_s // P                 # slots per partition (even)
    npair = c_s // 2
    pair_chunks = _pair_chunks(npair)
    pair_max = max(pair_chunks)

    nc = bacc.Bacc("TRN2", target_bir_lowering=False, debug=False,
                   num_swdge_queues=N_Q)

    x = nc.dram_tensor("x", [n_s, D], F16, kind="ExternalInput")
    catsp = nc.dram_tensor("catsp", [P, npair * 8], I16, kind="ExternalInput")
    table = nc.dram_tensor("table", [VOCAB, D], F16, kind="ExternalInput")
    y = nc.dram_tensor("y", [n_s, D], F16, kind="ExternalOutput")

    xr = x[:].rearrange("(p c) d -> p (c d)", p=P)     # [128, c_s*D]
    yr = y[:].rearrange("(p c) d -> p (c d)", p=P)

    # Issue the GPSIMD ucode library load BEFORE the TileContext so the
    # IRAM load overlaps Tile's own prologue barrier.
    from concourse.library_config import mlp
    nc.gpsimd.load_library(mlp)

    with tile.TileContext(nc) as tc:
        with (
            tc.tile_pool(name="const", bufs=1) as const_pool,
            tc.tile_pool(name="io", bufs=2) as io_pool,
            tc.tile_pool(name="gat", bufs=8) as gat_pool,
        ):
            cats_t = const_pool.tile([P, npair * 8], I16)
            nc.scalar.dma_start(out=cats_t[:], in_=catsp[:])

            # x in TWO big up-front DMAs (not one per chunk): a 9th+ HWDGE
            # DMA issues through the global 8-sem rotation and its
            # descriptors then starve behind the mid-run gather burst --
            # measured 30us-late x arrivals stalling the last muls. Two
            # ~4MB loads use early semaphores and queue their descriptors
            # before the gather flood starts.
            halves = []                 # (tile, pair_offset, pair_count)
            split = len(pair_chunks) // 2
            off = 0
            for h, hp in ((0, sum(pair_chunks[:split])),
                          (1, sum(pair_chunks[split:]))):
                xt = io_pool.tile([P, hp * 2 * D], F16, tag=f"xh{h}")
                nc.sync.dma_start(
                    out=xt[:], in_=xr[:, off * 2 * D:(off + hp) * 2 * D])
                halves.append((xt, off, hp))
                off += hp

            ppos = 0
            for ci, pch in enumerate(pair_chunks):
                # one gathered row per pair
                n_idx = pch * P
                g_t = gat_pool.tile([P, pair_max * D], F16, tag="g")
                nc.gpsimd.dma_gather(
                    out_ap=g_t[:, :pch * D].rearrange("p (t d) -> p t d",
                                                      t=pch),
                    in_ap=table[:],
                    idxs_ap=cats_t[:, ppos * 8:(ppos + pch) * 8],
                    num_idxs=n_idx,
                    num_idxs_reg=n_idx,
                    elem_size=D,
                    queue_num=ci % N_Q,
                )

                xt, h_off, h_np = next(
                    (t, o, n) for t, o, n in halves
                    if o <= ppos and ppos + pch <= o + n)
                loc = (ppos - h_off) * 2 * D
                xs = xt[:, loc:loc + pch * 2 * D]

                # x[p, pair, k, :] *= row[p, pair, :] broadcast over k (step 0)
                ga = g_t[:]
                x4 = bass.AP(xs.tensor, xs.offset,
                             [xs.ap[0], (2 * D, pch), (D, 2), (1, D)])
                g4 = bass.AP(ga.tensor, ga.offset,
                             [ga.ap[0], (D, pch), (0, 2), (1, D)])
                nc.vector.tensor_mul(out=x4, in0=x4, in1=g4)
                lo, hi = ppos * 2 * D, (ppos + pch) * 2 * D
                nc.scalar.dma_start(out=yr[:, lo:hi], in_=xs)
                ppos += pch

    nc.compile()
    return nc


_NC_CACHE = {}
_LAST_NS = N_S_MAX


def _get_nc(n_s=None):
    if n_s is None:
        n_s = _LAST_NS
    if n_s not in _NC_CACHE:
        _NC_CACHE[n_s] = _build_nc(n_s)
    return _NC_CACHE[n_s]


def _core_need(counts, npos):
    """Slots needed: positions + one pad per odd-count category."""
    return npos + int((counts & 1).sum())


def _balance_positions(cats):
    """Per-core global position lists, starting from contiguous blocks.

    Phase 1 -- parity pairing: for each category, cores where its count is
    odd are paired up and one position moves between each pair, making both
    counts even. Each such move removes two pad slots globally (the mover's
    core loses a position AND a pad; the receiver gains a position and
    loses a pad). Only categories with an odd GLOBAL count keep one odd
    core (~vocab/2 pads total instead of ~vocab/2 per core).

    Phase 2 -- need balancing: moving a position of a category that is odd
    in the source flips it even there (-2 slots) and odd at the destination
    (+2 slots), a pure transfer of slot need; use it to level the cores.
    """
    counts = np.stack([np.bincount(cats[i * N:(i + 1) * N],
                                   minlength=VOCAB) for i in range(N_CORES)])
    # per-core per-category global-position stacks (O(1) moves)
    by_cat = []
    for i in range(N_CORES):
        c = cats[i * N:(i + 1) * N]
        order = np.argsort(c, kind="stable") + i * N
        bnd = np.zeros(VOCAB + 1, dtype=np.int64)
        np.cumsum(counts[i], out=bnd[1:])
        by_cat.append([list(order[bnd[v]:bnd[v + 1]]) for v in range(VOCAB)])
    sizes = [N] * N_CORES

    def move(v, src, dst):
        by_cat[dst][v].append(by_cat[src][v].pop())
        counts[src][v] -= 1
        counts[dst][v] += 1
        sizes[src] -= 1
        sizes[dst] += 1

    for v in range(VOCAB):
        odd = [i for i in range(N_CORES) if counts[i][v] & 1]
        for k in range(0, len(odd) - 1, 2):
            a, b = odd[k], odd[k + 1]
            src, dst = (a, b) if sizes[a] >= sizes[b] else (b, a)
            move(v, src, dst)

    needs = [_core_need(counts[i], sizes[i]) for i in range(N_CORES)]
    for _ in range(2000):
        src = int(np.argmax(needs))
        dst = int(np.argmin(needs))
        if needs[src] - needs[dst] <= 2:
            break
        odd_src = np.flatnonzero(counts[src] & 1)
        if len(odd_src) == 0:
            break
        move(int(odd_src[0]), src, dst)
        needs[src] = _core_need(counts[src], sizes[src])
        needs[dst] = _core_need(counts[dst], sizes[dst])

    def ns_for(need):
        sp = -(-need // P)
        return (sp + (sp & 1)) * P

    pos = [np.array([p for v in range(VOCAB) for p in by_cat[i][v]],
                    dtype=np.int64) for i in range(N_CORES)]
    n_s = min(max(ns_for(n) for n in needs), N_S_MAX)
    return pos, n_s


def _sort_pad(c_local, n_s):
    """Sort a core's positions by category and pad so every within-partition
    pair of slots shares one category.

    Returns (slot_idx[n_s] int64 local indices with -1 for dummy slots,
             pair_cats[P, n_s//256] int16)."""
    npos = len(c_local)
    order = np.argsort(c_local, kind="stable")
    counts = np.bincount(c_local, minlength=VOCAB)
    padded = counts + (counts & 1)
    pstart = np.zeros(VOCAB + 1, dtype=np.int64)
    np.cumsum(padded, out=pstart[1:])
    bstart = np.zeros(VOCAB + 1, dtype=np.int64)
    np.cumsum(counts, out=bstart[1:])
    within = np.arange(npos, dtype=np.int64) - np.repeat(bstart[:-1], counts)
    slots = np.repeat(pstart[:-1], counts) + within
    slot_idx = np.full(n_s, -1, dtype=np.int64)
    slot_idx[slots] = order
    slot_cat = np.zeros(n_s, dtype=np.int16)
    slot_cat[:pstart[-1]] = np.repeat(
        np.arange(VOCAB, dtype=np.int16), padded)
    pair_cats = slot_cat[0::2].reshape(P, n_s // 256)
    return slot_idx, pair_cats


def _permute_pair_cats(pair_cats):
    """dma_gather idx stream: stream index s = pair_col*128 + p holds
    pair_cats[p, pair_col]; wrap (s at [s%16, s//16]) and replicate."""
    npairs = pair_cats.size
    a = np.ascontiguousarray(pair_cats.T).reshape(npairs)
    return np.ascontiguousarray(np.tile(a.reshape(npairs // 16, 16).T, (8, 1)))


def _shard_inputs(inputs, categories, emb_table):
    global _LAST_NS
    tab = np.array(emb_table, dtype=np.float16)
    tab[0, :] = np.float16(1.0)            # padding row -> multiplier 1.0
    cats = np.asarray(categories).reshape(B * S).astype(np.int64)
    xall = np.asarray(inputs, dtype=np.float16).reshape(B * S, D)
    pos, n_s = _balance_positions(cats)
    _LAST_NS = n_s
    in_maps = []
    shard_meta = []
    for i in range(N_CORES):
        slot_idx, pair_cats = _sort_pad(cats[pos[i]], n_s)
        valid = slot_idx >= 0
        slot_pos = np.full(n_s, -1, dtype=np.int64)
        slot_pos[valid] = pos[i][slot_idx[valid]]   # global position ids
        xdev = np.zeros((n_s, D), dtype=np.float16)
        xdev[valid] = xall[slot_pos[valid]]
        in_maps.append({"x": xdev, "catsp": _permute_pair_cats(pair_cats),
                        "table": tab})
        shard_meta.append((slot_pos, valid))
    return in_maps, shard_meta


def kernel(inputs, categories, mask_positions=None, emb_table=None, **_):
    """Full (unsharded) inputs in, full output out. mask_positions unused."""
    in_maps, shard_meta = _shard_inputs(inputs, categories, emb_table)
    nc = _get_nc(_LAST_NS)
    res = run_bass_kernel_spmd(nc, in_maps, list(range(N_CORES)))
    out = np.empty((B * S, D), dtype=np.float32)
    for i in range(N_CORES):
        slot_pos, valid = shard_meta[i]
        ydev = res.results[i]["y"].reshape(_LAST_NS, D)
        out[slot_pos[valid]] = ydev[valid].astype(np.float32)
    return out.reshape(B, S, D)


# revision 36
# speedup vs baseline: 1.0750x; 1.0117x over previous
"""Trainium2 Bass kernel for nn_CategoryMultiplier.

out[b, s, :] = inputs[b, s, :] * (emb_table[categories[b, s]] if
               categories[b, s] != 0 else 1.0)

Sharding: data parallel, 8192 positions per core. Cores start from
contiguous 16-batch blocks; a few positions are then exchanged between
cores to balance padding (see below) -- the host fully controls the
position->slot mapping, so this is pure layout prep.

Precision: the grading gate is rel_err < 2e-2; fp16 end-to-end keeps the
max relative error at ~7e-4 while halving every HBM stream. Host converts
f32 -> fp16 in and back out.

Category-sorted pairing (the big byte saver): the host sorts each core's
positions by category, so equal-category runs become contiguous slots,
and pads odd runs so every within-partition PAIR of slots shares one
category. The kernel gathers ONE table row per pair and the DVE multiply
broadcasts each row over its pair with a stride-0 AP dim. Dummy slots
carry x = 0 and are dropped on the host-side unpermute.

Adaptive slot count: the padded slot count N_S depends on the input (one
pad slot per odd-count category, plus partition alignment). The kernel is
compiled lazily for the exact N_S the inputs need (cached per size)
instead of the 9216 worst case. Cross-core balancing first moves single
positions of categories that are odd in both the source and destination
core: such a move cuts the source's slot need by 2 and leaves the
destination's unchanged, pulling outlier cores under the next 256-slot
alignment boundary. Typical result: N_S = 8704 (68 slots/partition),
i.e. 23.1MB of DMA per core -- the 16 DMA engines at ~22.3GB/s each are
the roofline.

Chunking: at most 8 SWDGE gathers (matches Tile's global 8-sem SWDGE pool
so sem rotation never kicks in) and 17 HWDGE DMAs. Head taper ([2,4,5...]
pairs) starts the mul->store chain right after the ~19us GPSIMD library
load. Gathers rotate queue_num 0..3: each SWDGE queue is served by its
own Q7 core pair (ucode dispatches on cpu_id/2 == queue_num), so four
descriptor generators run concurrently. x-loads stay alone on sync and
y-stores alone on scalar: a store (which waits on compute) placed in the
prefetch engine's in-order queue would head-of-line-block the x stream.

Padding rows (category 0 -> multiplier 1.0): baked into the host fp16
table copy (row 0 = ones); index 0 is semantically dead.
"""

import numpy as np

import concourse.bass as bass
import concourse.bacc as bacc
import concourse.mybir as mybir
import concourse.tile as tile
from concourse.bass_utils import run_bass_kernel_spmd

# Problem shape (hardcoded per harness contract).
B, S, D = 128, 512, 512
VOCAB = 1000
N_CORES = 8
B_LOC = B // N_CORES            # 16 batches per core
N = B_LOC * S                   # 8192 positions per core
P = 128                         # SBUF partitions
N_S_MAX = 9216                  # worst case: 8192 + 1000 odd cats, aligned
N_Q = 4                         # SWDGE queues / Q7 pairs used for gathers

F16 = mybir.dt.float16
I16 = mybir.dt.int16


K_GRP = 4                       # slots per gathered row (quad grouping)


def _grp_chunks(ngrp):
    """Head-tapered chunk list: [1, 2, then <=3s], at most 8 chunks."""
    rest = ngrp - 3
    assert 0 < rest <= 15, ngrp
    n_tail = -(-rest // 3)
    chunks = [1, 2]
    for i in range(n_tail):
        chunks.append(rest // n_tail + (1 if i < rest % n_tail else 0))
    assert sum(chunks) == ngrp and len(chunks) <= 8 and max(chunks) <= 3
    return chunks


def _build_nc(n_s):
    c_s = n_s // P                 # slots per partition (multiple of K_GRP)
    ngrp = c_s // K_GRP
    grp_chunks = _grp_chunks(ngrp)

    nc = bacc.Bacc("TRN2", target_bir_lowering=False, debug=False,
                   num_swdge_queues=N_Q)

    x = nc.dram_tensor("x", [n_s, D], F16, kind="ExternalInput")
    catsp = nc.dram_tensor("catsp", [P, ngrp * P // 16], I16,
                           kind="ExternalInput")
    table = nc.dram_tensor("table", [VOCAB, D], F16, kind="ExternalInput")
    y = nc.dram_tensor("y", [n_s, D], F16, kind="ExternalOutput")

    xr = x[:].rearrange("(p c) d -> p (c d)", p=P)     # [128, c_s*D]
    yr = y[:].rearrange("(p c) d -> p (c d)", p=P)

    # Issue the GPSIMD ucode library load BEFORE the TileContext so the
    # IRAM load overlaps Tile's own prologue barrier.
    from concourse.library_config import mlp
    nc.gpsimd.load_library(mlp)

    with tile.TileContext(nc) as tc:
        with (
            tc.tile_pool(name="const", bufs=1) as const_pool,
            tc.tile_pool(name="io", bufs=2) as io_pool,
            tc.tile_pool(name="gat", bufs=8) as gat_pool,
        ):
            cats_t = const_pool.tile([P, ngrp * P // 16], I16)
            nc.scalar.dma_start(out=cats_t[:], in_=catsp[:])

            # x in TWO big up-front DMAs (not one per chunk): a 9th+ HWDGE
            # DMA issues through the global 8-sem rotation and its
            # descriptors then starve behind the mid-run gather burst --
            # measured 30us-late x arrivals stalling the last muls. Two
            # ~4MB loads use early semaphores and queue their descriptors
            # before the gather flood starts.
            halves = []                 # (tile, grp_offset, grp_count)
            split = len(grp_chunks) // 2
            off = 0
            for h, hg in ((0, sum(grp_chunks[:split])),
                          (1, sum(grp_chunks[split:]))):
                xt = io_pool.tile([P, hg * K_GRP * D], F16, tag=f"xh{h}")
                nc.sync.dma_start(
                    out=xt[:],
                    in_=xr[:, off * K_GRP * D:(off + hg) * K_GRP * D])
                halves.append((xt, off, hg))
                off += hg

            gpos = 0
            for ci, gch in enumerate(grp_chunks):
                # one gathered row per quad group
                n_idx = gch * P
                g_t = gat_pool.tile([P, max(grp_chunks) * D], F16, tag="g")
                nc.gpsimd.dma_gather(
                    out_ap=g_t[:, :gch * D].rearrange("p (t d) -> p t d",
                                                      t=gch),
                    in_ap=table[:],
                    idxs_ap=cats_t[:, gpos * 8:(gpos + gch) * 8],
                    num_idxs=n_idx,
                    num_idxs_reg=n_idx,
                    elem_size=D,
                    queue_num=ci % N_Q,
                )

                xt, h_off, h_ng = next(
                    (t, o, n) for t, o, n in halves
                    if o <= gpos and gpos + gch <= o + n)
                loc = (gpos - h_off) * K_GRP * D
                xs = xt[:, loc:loc + gch * K_GRP * D]

                # x[p, grp, k, :] *= row[p, grp, :] broadcast over k (step 0)
                ga = g_t[:]
                x4 = bass.AP(xs.tensor, xs.offset,
                             [xs.ap[0], (K_GRP * D, gch), (D, K_GRP), (1, D)])
                g4 = bass.AP(ga.tensor, ga.offset,
                             [ga.ap[0], (D, gch), (0, K_GRP), (1, D)])
                nc.vector.tensor_mul(out=x4, in0=x4, in1=g4)
                lo, hi = gpos * K_GRP * D, (gpos + gch) * K_GRP * D
                nc.scalar.dma_start(out=yr[:, lo:hi], in_=xs)
                gpos += gch

    nc.compile()
    return nc


_NC_CACHE = {}
_LAST_NS = N_S_MAX


def _get_nc(n_s=None):
    if n_s is None:
        n_s = _LAST_NS
    if n_s not in _NC_CACHE:
        _NC_CACHE[n_s] = _build_nc(n_s)
    return _NC_CACHE[n_s]


def _core_need(counts, npos):
    """Slots needed: positions + pad to a multiple of K_GRP per category."""
    return npos + int(((-counts) % K_GRP).sum())


def _balance_positions(cats):
    """Per-core global position lists, starting from contiguous blocks.

    Phase 1 -- parity pairing: for each category, cores where its count is
    odd are paired up and one position moves between each pair, making both
    counts even. Each such move removes two pad slots globally (the mover's
    core loses a position AND a pad; the receiver gains a position and
    loses a pad). Only categories with an odd GLOBAL count keep one odd
    core (~vocab/2 pads total instead of ~vocab/2 per core).

    Phase 2 -- need balancing: moving a position of a category that is odd
    in the source flips it even there (-2 slots) and odd at the destination
    (+2 slots), a pure transfer of slot need; use it to level the cores.
    """
    counts = np.stack([np.bincount(cats[i * N:(i + 1) * N],
                                   minlength=VOCAB) for i in range(N_CORES)])
    # per-core per-category global-position stacks (O(1) moves)
    by_cat = []
    for i in range(N_CORES):
        c = cats[i * N:(i + 1) * N]
        order = np.argsort(c, kind="stable") + i * N
        bnd = np.zeros(VOCAB + 1, dtype=np.int64)
        np.cumsum(counts[i], out=bnd[1:])
        by_cat.append([list(order[bnd[v]:bnd[v + 1]]) for v in range(VOCAB)])
    sizes = [N] * N_CORES

    def move(v, src, dst):
        by_cat[dst][v].append(by_cat[src][v].pop())
        counts[src][v] -= 1
        counts[dst][v] += 1
        sizes[src] -= 1
        sizes[dst] += 1

    # Phase 1: consolidate each category's mod-K residues onto the core that
    # already holds the largest residue — every other core's residue moves
    # there, making those cores' counts exact multiples of K (zero pad).
    for v in range(VOCAB):
        res = [counts[i][v] % K_GRP for i in range(N_CORES)]
        holder = int(np.argmax(res))
        if res[holder] == 0:
            continue
        for i in range(N_CORES):
            if i != holder and res[i]:
                for _ in range(res[i]):
                    move(v, i, holder)

    # Phase 2: level the cores. Moving a category's full residue (r in
    # 1..K-1) transfers exactly K slots of need (src: -r positions -(K-r)
    # pads; dst: +r positions +(K-r) pads). If the max core has no residue,
    # move K positions of any populous category (pure size transfer).
    needs = [_core_need(counts[i], sizes[i]) for i in range(N_CORES)]
    for _ in range(2000):
        src = int(np.argmax(needs))
        dst = int(np.argmin(needs))
        if needs[src] - needs[dst] <= K_GRP:
            break
        res_src = counts[src] % K_GRP
        cand = np.flatnonzero(res_src)
        if len(cand):
            v = int(cand[0])
            for _ in range(int(res_src[v])):
                move(v, src, dst)
        else:
            big = np.flatnonzero(counts[src] >= K_GRP)
            if len(big) == 0:
                break
            v = int(big[0])
            for _ in range(K_GRP):
                move(v, src, dst)
        needs[src] = _core_need(counts[src], sizes[src])
        needs[dst] = _core_need(counts[dst], sizes[dst])

    def ns_for(need):
        sp = -(-need // P)
        return -(-sp // K_GRP) * K_GRP * P

    pos = [np.array([p for v in range(VOCAB) for p in by_cat[i][v]],
                    dtype=np.int64) for i in range(N_CORES)]
    n_s = min(max(ns_for(n) for n in needs), N_S_MAX)
    return pos, n_s


def _sort_pad(c_local, n_s):
    """Sort a core's positions by category and pad so every within-partition
    group of K_GRP slots shares one category.

    Returns (slot_idx[n_s] int64 local indices with -1 for dummy slots,
             grp_cats[P, n_s//(128*K_GRP)] int16)."""
    npos = len(c_local)
    order = np.argsort(c_local, kind="stable")
    counts = np.bincount(c_local, minlength=VOCAB)
    padded = counts + (-counts) % K_GRP
    pstart = np.zeros(VOCAB + 1, dtype=np.int64)
    np.cumsum(padded, out=pstart[1:])
    assert pstart[-1] <= n_s, (pstart[-1], n_s)
    bstart = np.zeros(VOCAB + 1, dtype=np.int64)
    np.cumsum(counts, out=bstart[1:])
    within = np.arange(npos, dtype=np.int64) - np.repeat(bstart[:-1], counts)
    slots = np.repeat(pstart[:-1], counts) + within
    slot_idx = np.full(n_s, -1, dtype=np.int64)
    slot_idx[slots] = order
    slot_cat = np.zeros(n_s, dtype=np.int16)
    slot_cat[:pstart[-1]] = np.repeat(
        np.arange(VOCAB, dtype=np.int16), padded)
    grp_cats = slot_cat[0::K_GRP].reshape(P, n_s // (P * K_GRP))
    return slot_idx, grp_cats


def _permute_pair_cats(pair_cats):
    """dma_gather idx stream: stream index s = pair_col*128 + p holds
    pair_cats[p, pair_col]; wrap (s at [s%16, s//16]) and replicate."""
    npairs = pair_cats.size
    a = np.ascontiguousarray(pair_cats.T).reshape(npairs)
    return np.ascontiguousarray(np.tile(a.reshape(npairs // 16, 16).T, (8, 1)))


def _shard_inputs(inputs, categories, emb_table):
    global _LAST_NS
    tab = np.array(emb_table, dtype=np.float16)
    tab[0, :] = np.float16(1.0)            # padding row -> multiplier 1.0
    cats = np.asarray(categories).reshape(B * S).astype(np.int64)
    xall = np.asarray(inputs, dtype=np.float16).reshape(B * S, D)
    pos, n_s = _balance_positions(cats)
    _LAST_NS = n_s
    in_maps = []
    shard_meta = []
    for i in range(N_CORES):
        slot_idx, pair_cats = _sort_pad(cats[pos[i]], n_s)
        valid = slot_idx >= 0
        slot_pos = np.full(n_s, -1, dtype=np.int64)
        slot_pos[valid] = pos[i][slot_idx[valid]]   # global position ids
        xdev = np.zeros((n_s, D), dtype=np.float16)
        xdev[valid] = xall[slot_pos[valid]]
        in_maps.append({"x": xdev, "catsp": _permute_pair_cats(pair_cats),
                        "table": tab})
        shard_meta.append((slot_pos, valid))
    return in_maps, shard_meta


def kernel(inputs, categories, mask_positions=None, emb_table=None, **_):
    """Full (unsharded) inputs in, full output out. mask_positions unused."""
    in_maps, shard_meta = _shard_inputs(inputs, categories, emb_table)
    nc = _get_nc(_LAST_NS)
    res = run_bass_kernel_spmd(nc, in_maps, list(range(N_CORES)))
    out = np.empty((B * S, D), dtype=np.float32)
    for i in range(N_CORES):
        slot_pos, valid = shard_meta[i]
        ydev = res.results[i]["y"].reshape(_LAST_NS, D)
        out[slot_pos[valid]] = ydev[valid].astype(np.float32)
    return out.reshape(B, S, D)
